# revision 1
# baseline (speedup 1.0000x reference)
"""CrossModalHGNN Trainium2 kernel — 8 NeuronCores, Bass/Tile.

Algorithm (exactly mirrors the reference, without materializing the
8192x8192 H):  H = 2I + [[0, Wi^T], [Wt^T, 0]] where Wt (Bi x Bt) and
Wi (Bt x Bi) hold the mutual-top-8 softmax weights.

Per core c (owns image rows Ic = [512c,512c+512) and text rows Tc):
  A) S-block  = fi_n[Ic] @ ft_n^T / tau   (512x4096 fp32 on PE)
     St-block = ft_n[Tc] @ fi_n^T / tau
     row top-8 via nc.vector.max -> per-row 8th-largest thresholds.
     AllGather thresholds (tiny), then mutual mask
        M[i,j] = S[i,j] >= max(thrA_i, thrB_j)
     second top-8 == M plus the lowest-index non-mutual positions
     (all non-mutual entries tie at NEG), found with a prefix-scan.
     Softmax over the selected entries (shift by row max — invariant).
     -> Wt/Wi rows in fp16, plus transposes via PE.
  B) 2 HGNN layers, fp16 matmuls (err ~3e-4):
     Xe = (2*X0 + W @ X0_other) * de_inv          (local rows)
     X1 = (2*Xe + W^T-aggregation) * dv_is         (ReduceScatter)
     X  = LN(X + gelu(X1 @ Wfc^T))                 (local rows)
     AllGather X between layers.  Final per-modality LN -> outputs.
"""

import numpy as np

import concourse.bass as bass
import concourse.mybir as mybir
from concourse.tile import TileContext
from concourse import bass_utils

dt = mybir.dt
AF = mybir.ActivationFunctionType
OP = mybir.AluOpType
ts = bass.ts

NCORES = 8
B = 4096          # images = texts
D = 512           # feature dim
CH = 512          # rows per core per modality
TOPK = 8
TAU = 0.07
SELF_LOOP = 2.0
EPS_DEG = 1e-6
LN_EPS = 1e-5
LAYERS = 2
NT = B // 128     # 32 tiles over 4096 rows
NTC = CH // 128   # 4 tiles over a 512-row chunk
ND = D // 128     # 4 tiles over feature dim


def _fixup_multi_waits(nc, max_waits=1):
    """This walrus build accepts at most one sync wait per instruction;
    hoist extras onto preceding EventSemaphore carriers (same engine)."""
    n_fixed = 0
    for f in nc.m.functions:
        for bb in f.blocks:
            new_insts = []
            for inst in bb.instructions:
                si = inst.sync_info
                waits = list(si.on_wait) if (si is not None and si.on_wait) else []
                if len(waits) > max_waits:
                    keep = waits[-max_waits:]
                    for w in waits[:-max_waits]:
                        carrier = mybir.InstEventSemaphore(
                            name=f"{inst.name}_wc{n_fixed}",
                            engine=inst.engine, ins=[], outs=[],
                            sync_info=mybir.SyncInfo(on_wait=[w], on_update=[]),
                        )
                        nc.register_instruction(carrier)
                        new_insts.append(carrier)
                        n_fixed += 1
                    inst.sync_info = mybir.SyncInfo(
                        on_wait=list(keep), on_update=list(si.on_update))
                new_insts.append(inst)
            bb.instructions = new_insts
    return n_fixed


def _make_identity(nc, identity):
    nc.gpsimd.memset(identity, 0.0)
    nc.gpsimd.affine_select(
        out=identity, in_=identity, compare_op=OP.not_equal, fill=1.0,
        base=0, pattern=[[-1, identity.shape[0]]], channel_multiplier=1)


def build_nc(debug=False):
    nc = bass.Bass()
    f32, f16, bf16 = dt.float32, dt.float16, dt.bfloat16

    # ---------------- I/O ----------------
    f_img = nc.dram_tensor("f_img", [B, D], f32, kind="ExternalInput")
    f_txt = nc.dram_tensor("f_txt", [B, D], f32, kind="ExternalInput")
    f_img_my = nc.dram_tensor("f_img_my", [CH, D], f32, kind="ExternalInput")
    f_txt_my = nc.dram_tensor("f_txt_my", [CH, D], f32, kind="ExternalInput")
    Wfc = nc.dram_tensor("Wfc", [LAYERS, D, D], f32, kind="ExternalInput")
    ln_g = nc.dram_tensor("ln_g", [LAYERS, D], f32, kind="ExternalInput")
    ln_b = nc.dram_tensor("ln_b", [LAYERS, D], f32, kind="ExternalInput")
    ln_img_g = nc.dram_tensor("ln_img_g", [D], f32, kind="ExternalInput")
    ln_img_b = nc.dram_tensor("ln_img_b", [D], f32, kind="ExternalInput")
    ln_txt_g = nc.dram_tensor("ln_txt_g", [D], f32, kind="ExternalInput")
    ln_txt_b = nc.dram_tensor("ln_txt_b", [D], f32, kind="ExternalInput")
    out_img_my = nc.dram_tensor("out_img_my", [CH, D], f32, kind="ExternalOutput")
    out_txt_my = nc.dram_tensor("out_txt_my", [CH, D], f32, kind="ExternalOutput")
    if debug:
        dbg_wt = nc.dram_tensor("dbg_wt", [CH, B], f16, kind="ExternalOutput")
        dbg_wi = nc.dram_tensor("dbg_wi", [CH, B], f16, kind="ExternalOutput")
        dbg_thr = nc.dram_tensor("dbg_thr", [B, 2], f32, kind="ExternalOutput")
        dbg_dv = nc.dram_tensor("dbg_dv", [128, 64], f32, kind="ExternalOutput")
        dbg_de = nc.dram_tensor("dbg_de", [128, 8], f32, kind="ExternalOutput")
        dbg_x1 = nc.dram_tensor("dbg_x1", [2 * CH, D], f16, kind="ExternalOutput")
        dbg_xe = nc.dram_tensor("dbg_xe", [2 * CH, D], f16, kind="ExternalOutput")
        dbg_S = nc.dram_tensor("dbg_S", [CH, B], f32, kind="ExternalOutput")
        dbg_M = nc.dram_tensor("dbg_M", [CH, B], f16, kind="ExternalOutput")
        dbg_sel = nc.dram_tensor("dbg_sel", [CH, B], f16, kind="ExternalOutput")
        dbg_mcnt = nc.dram_tensor("dbg_mcnt", [CH, 1], f32, kind="ExternalOutput")

    # ---------------- internal DRAM ----------------
    S_dram = nc.dram_tensor("S_spill", [CH, B], f32)       # my image-side S rows
    St_dram = nc.dram_tensor("St_spill", [CH, B], f32)     # my text-side S rows
    Wt_dram = nc.dram_tensor("Wt_d", [CH, B], f16)         # Wt rows (img edges)
    Wi_dram = nc.dram_tensor("Wi_d", [CH, B], f16)
    WtT_dram = nc.dram_tensor("WtT_d", [B, CH], f16)       # (txt, my img)
    WiT_dram = nc.dram_tensor("WiT_d", [B, CH], f16)       # (img, my txt)
    thr_in = nc.dram_tensor("thr_in", [CH, 2], f32)
    thr_out = nc.dram_tensor("thr_out", [B, 2], f32, addr_space="Shared")
    dv_ar_in = nc.dram_tensor("dv_ar_in", [128, 64], f32)
    dv_ar_out = nc.dram_tensor("dv_ar_out", [128, 64], f32, addr_space="Shared")
    dv_rs_in = nc.dram_tensor("dv_rs_in", [NCORES, 128, 8], f32)
    dv_rs_out = nc.dram_tensor("dv_rs_out", [1, 128, 8], f32)
    x1p = [nc.dram_tensor(f"x1p_{l}", [NCORES, 2 * CH, D], f16) for l in range(LAYERS)]
    x1rs = [nc.dram_tensor(f"x1rs_{l}", [1, 2 * CH, D], f16)
            for l in range(LAYERS)]
    xag_in = nc.dram_tensor("xag_in", [2 * CH, D], f16)
    xag_out = nc.dram_tensor("xag_out", [NCORES * 2 * CH, D], f16, addr_space="Shared")

    RG = [list(range(NCORES))]

    with TileContext(nc) as tc:
        # ---- persistent small constants / vectors ----
        const = tc.alloc_tile_pool(name="const", bufs=1)
        id_f32 = const.tile([128, 128], f32, name="idf32", tag="idf32")
        id_f16 = const.tile([128, 128], f16, name="idf16", tag="idf16")
        _make_identity(nc, id_f32[:])
        _make_identity(nc, id_f16[:])

        # thresholds / rowmax collected during A2  (col j = row-tile j)
        thrA_my = const.tile([128, NTC], f32, name="thrA", tag="thrA")    # img side
        thrB_my = const.tile([128, NTC], f32, name="thrB", tag="thrB")    # txt side
        nmaxA = const.tile([128, NTC], f32, name="nmaxA", tag="nmaxA")     # -rowmax img
        nmaxB = const.tile([128, NTC], f32, name="nmaxB", tag="nmaxB")
        de_inv_i = const.tile([128, NTC], f32, name="deinvi", tag="deinvi")  # img edges (mine)
        de_inv_t = const.tile([128, NTC], f32, name="deinvt", tag="deinvt")
        colp_img = const.tile([128, NT], f32, name="colpi", tag="colpi")   # colsum(Wi) partial
        colp_txt = const.tile([128, NT], f32, name="colpt", tag="colpt")   # colsum(Wt) partial
        dv_is_full = const.tile([128, 64], f32, name="dvisf", tag="dvisf")  # cols 0-31 img, 32-63 txt
        dv_is_my = const.tile([128, 8], f32, name="dvism", tag="dvism")     # cols 0-3 img, 4-7 txt

        # FC weights transposed, fp16: WT[l][k] = (128 in x 512 out)
        wt_fc = [[const.tile([128, D], f16, name=f"wfc{l}_{k}", tag=f"wfc{l}_{k}") for k in range(ND)]
                 for l in range(LAYERS)]


        # ---------- P0: FC weight transpose ----------
        with tc.tile_pool(name="p0", bufs=2) as p0, \
             tc.tile_pool(name="p0ps", bufs=4, space="PSUM") as p0ps:
            for l in range(LAYERS):
                wrow16 = []
                for r in range(ND):
                    st32 = p0.tile([128, D], f32, name="wld", tag="wld")
                    nc.sync.dma_start(st32[:], Wfc[l, ts(r, 128), :])
                    st16 = p0.tile([128, D], f16, name="wld16", tag="wld16", bufs=4)
                    nc.scalar.copy(st16[:], st32[:])
                    wrow16.append(st16)
                # wt_fc[l][k][:, o] = Wfc[l][o, k]  (transpose)
                for k in range(ND):
                    ps = p0ps.tile([128, D], f16, name="wtp", tag="wtp")
                    for r in range(ND):
                        nc.tensor.transpose(
                            ps[:, ts(r, 128)], wrow16[r][:, ts(k, 128)], id_f16[:])
                    nc.scalar.copy(wt_fc[l][k][:], ps[:])

        # ---------- A0: normalized transposed features ----------
        # chunk transposes live through both similarity phases
        chunkT = tc.alloc_tile_pool(name="chunkT", bufs=1)
        fiT_my = [chunkT.tile([128, CH], f32, name=f"fiTm{k}", tag=f"fiTm{k}") for k in range(ND)]
        ftT_my = [chunkT.tile([128, CH], f32, name=f"ftTm{k}", tag=f"ftTm{k}") for k in range(ND)]
        featTb = tc.alloc_tile_pool(name="featTb", bufs=1)
        fiT = [featTb.tile([128, B], f32, name=f"fiT{k}", tag=f"fiT{k}") for k in range(ND)]
        featT = tc.alloc_tile_pool(name="featT", bufs=1)
        ftT = [featT.tile([128, B], f32, name=f"ftT{k}", tag=f"ftT{k}") for k in range(ND)]

        def normalize_transpose(src_dram, n_tiles, outT, pool, psp):
            # src (n_tiles*128 x D) -> outT[k] (128 x n_tiles*128), L2-normalized rows
            for tgrp in range(n_tiles // 4):      # groups of 4 row-tiles
                fn_grp = []
                for j in range(4):
                    t = tgrp * 4 + j
                    raw = pool.tile([128, D], f32, name="rawf", tag="rawf")
                    nc.sync.dma_start(raw[:], src_dram[ts(t, 128), :])
                    # exact squares + pairwise tree-sum (sequential accum is
                    # ~1e-6 rel; selections need the fp32 noise floor)
                    sq = pool.tile([128, D], f32, name="sqscr", tag="sqscr")
                    nc.vector.tensor_mul(sq[:], raw[:], raw[:])
                    tsum = pool.tile([128, D // 2], f32, name="tsum", tag="tsum", bufs=2)
                    nc.vector.tensor_add(tsum[:, :], sq[:, 0:D // 2], sq[:, D // 2:D])
                    w = D // 4
                    while w >= 1:
                        nc.vector.tensor_add(tsum[:, 0:w], tsum[:, 0:w], tsum[:, w:2 * w])
                        w //= 2
                    ssq = tsum[:, 0:1]
                    nrm = pool.tile([128, 1], f32, name="nrm", tag="nrm")
                    nc.scalar.activation(nrm[:], ssq[:], AF.Sqrt)
                    r0 = pool.tile([128, 1], f32, name="r0", tag="r0")
                    nc.vector.reciprocal(r0[:], nrm[:])
                    # Newton step: rinv = r0*(1.5 - 0.5*ssq*r0^2)  (ACT Sqrt is
                    # only ~7e-6 accurate; selection needs fp32-exact norms)
                    rr = pool.tile([128, 1], f32, name="rr", tag="rr")
                    nc.vector.tensor_mul(rr[:], r0[:], r0[:])
                    t2 = pool.tile([128, 1], f32, name="t2n", tag="t2n")
                    nc.vector.tensor_mul(t2[:], rr[:], ssq)
                    t3 = pool.tile([128, 1], f32, name="t3n", tag="t3n")
                    nc.vector.tensor_scalar(t3[:], t2[:], -0.5, 1.5, op0=OP.mult, op1=OP.add)
                    rinv = pool.tile([128, 1], f32, name="rinv", tag="rinv")
                    nc.vector.tensor_mul(rinv[:], r0[:], t3[:])
                    fn = pool.tile([128, D], f32, name="fn", tag="fn", bufs=6)
                    nc.scalar.activation(fn[:], raw[:], AF.Copy, scale=rinv[:])
                    fn_grp.append((t, fn))
                for k in range(ND):
                    ps = psp.tile([128, 512], f32, name="tps", tag="tps")
                    for j, (t, fn) in enumerate(fn_grp):
                        nc.tensor.transpose(
                            ps[:, ts(j, 128)], fn[:, ts(k, 128)], id_f32[:])
                    t0 = fn_grp[0][0]
                    nc.vector.tensor_copy(outT[k][:, t0 * 128:(t0 + 4) * 128], ps[:])

        with tc.tile_pool(name="a0", bufs=2) as a0, \
             tc.tile_pool(name="a0ps", bufs=8, space="PSUM") as a0ps:
            normalize_transpose(f_txt, NT, ftT, a0, a0ps)
            normalize_transpose(f_img_my, NTC, fiT_my, a0, a0ps)
            normalize_transpose(f_txt_my, NTC, ftT_my, a0, a0ps)

        # ---------- A2: similarity blocks, row top-8, spill ----------
        def sim_block(kxm_my, kxnT, spill, thr_tile, nmax_tile, pool, psp):
            for m in range(NTC):
                srow = pool.tile([128, B], f32, name="srow", tag="srow", bufs=1)
                for n in range(8):
                    ps = psp.tile([128, 512], f32, name="sps", tag="sps")
                    for k in range(ND):
                        nc.tensor.matmul(
                            ps[:], kxm_my[k][:, ts(m, 128)], kxnT[k][:, ts(n, 512)],
                            start=(k == 0), stop=(k == ND - 1))
                    nc.scalar.activation(srow[:, ts(n, 512)], ps[:], AF.Copy,
                                         scale=1.0 / TAU)
                m8 = pool.tile([128, 8], f32, name="m8", tag="m8")
                nc.vector.max(out=m8[:], in_=srow[:])
                nc.vector.tensor_copy(thr_tile[:, m:m + 1], m8[:, 7:8])
                nc.vector.tensor_scalar(nmax_tile[:, m:m + 1], m8[:, 0:1],
                                        -1.0, None, op0=OP.mult)
                nc.sync.dma_start(spill[ts(m, 128), :], srow[:])

        # build fiT up front so it overlaps the row-block matmuls
        with tc.tile_pool(name="a0b", bufs=2) as a0b, \
             tc.tile_pool(name="a0bps", bufs=4, space="PSUM") as a0bps:
            normalize_transpose(f_img, NT, fiT, a0b, a0bps)

        with tc.tile_pool(name="a2", bufs=2) as a2, \
             tc.tile_pool(name="a2ps", bufs=8, space="PSUM") as a2ps:
            sim_block(fiT_my, ftT, S_dram, thrA_my, nmaxA, a2, a2ps)
        featT.release()   # free ftT

        # text-side block: SAME operand roles (image stationary), then exact
        # PE transpose, so St[j,i] is bitwise equal to S[i,j] and the top-8
        # defining elements tie identically across cores.
        strows = tc.alloc_tile_pool(name="strows", bufs=1)
        st_rows = [strows.tile([128, B], f32, name=f"strow{t}", tag=f"strow{t}")
                   for t in range(NTC)]
        with tc.tile_pool(name="a2b", bufs=2) as a2b, \
             tc.tile_pool(name="a2bps", bufs=1, space="PSUM") as a2bps:
            pst = [a2bps.tile([128, 512], f32, name=f"pst{t}", tag=f"pst{t}", bufs=1)
                   for t in range(NTC)]
            for m in range(NT):
                ps = a2bps.tile([128, 512], f32, name="cps", tag="cps", bufs=2)
                for k in range(ND):
                    nc.tensor.matmul(ps[:], fiT[k][:, ts(m, 128)], ftT_my[k][:],
                                     start=(k == 0), stop=(k == ND - 1))
                stage = a2b.tile([128, 512], f32, name="cstg", tag="cstg", bufs=2)
                nc.scalar.activation(stage[:], ps[:], AF.Copy, scale=1.0 / TAU)
                for t in range(NTC):
                    nc.tensor.transpose(pst[t][:, ts(m % 4, 128)],
                                        stage[:, ts(t, 128)], id_f32[:])
                if m % 4 == 3:
                    for t in range(NTC):
                        nc.vector.tensor_copy(
                            st_rows[t][:, (m // 4) * 512:(m // 4 + 1) * 512], pst[t][:])
            for t in range(NTC):
                m8b = a2b.tile([128, 8], f32, name="m8b", tag="m8b")
                nc.vector.max(out=m8b[:], in_=st_rows[t][:])
                nc.vector.tensor_copy(thrB_my[:, t:t + 1], m8b[:, 7:8])
                nc.vector.tensor_scalar(nmaxB[:, t:t + 1], m8b[:, 0:1],
                                        -1.0, None, op0=OP.mult)
                nc.sync.dma_start(St_dram[ts(t, 128), :], st_rows[t][:])
        strows.release()
        featTb.release()
        chunkT.release()

        # ---------- A3: threshold allgather ----------
        with tc.tile_pool(name="a3", bufs=1) as a3:
            pk = a3.tile([128, 2 * NTC], f32, name="thrpk", tag="thrpk")
            for m in range(NTC):
                nc.vector.tensor_copy(pk[:, 2 * m:2 * m + 1], thrA_my[:, m:m + 1])
                nc.vector.tensor_copy(pk[:, 2 * m + 1:2 * m + 2], thrB_my[:, m:m + 1])
            # thr_in[(m*128+p), 0:2] = pk[p, 2m:2m+2]
            for m in range(NTC):
                nc.sync.dma_start(thr_in[ts(m, 128), :], pk[:, 2 * m:2 * m + 2])
            nc.gpsimd.collective_compute(
                "AllGather", OP.bypass, replica_groups=RG,
                ins=[thr_in[:, :].opt()], outs=[thr_out[:, :].opt()])

        # one shared bcast tile of full thresholds along the free axis
        thrb_pool = tc.alloc_tile_pool(name="thrb", bufs=2)
        ones_bf = thrb_pool.tile([128, B], bf16, name="ones", tag="ones", bufs=1)
        nc.gpsimd.memset(ones_bf[:], 1.0)
        def load_thr_bc(col):
            thr_bc = thrb_pool.tile([128, B], f32, name="thrbc", tag="thrbc", bufs=2)
            nc.sync.dma_start(
                thr_bc[:], thr_out[:, col:col + 1].rearrange("a b -> b a").to_broadcast((128, B)))
            return thr_bc

        # ---------- A4 + A6: masks, softmax, W rows, transposes ----------
        def build_W(spill, thr_row_tile, nmax_tile, thr_col_bc, w_dram, wT_dram,
                    de_tile, colp_tile, pool, psp, dump=False):
            wrows = []
            for m in range(NTC):
                srow = pool.tile([128, B], f32, name="mrow", tag="mrow", bufs=2)
                nc.sync.dma_start(srow[:], spill[ts(m, 128), :])
                M = pool.tile([128, B], bf16, name="Mm", tag="Mm", bufs=2)
                mcnt = pool.tile([128, 1], f32, name="mcnt", tag="mcnt")
                # M = (max(thr_col, thr_row) <= S); mcnt = sum(M)
                nc.vector.scalar_tensor_tensor(
                    out=M[:], in0=thr_col_bc[:], scalar=thr_row_tile[:, m:m + 1],
                    in1=srow[:], op0=OP.max, op1=OP.is_le, accum_out=mcnt[:])
                pad_k = pool.tile([128, 1], f32, name="padk", tag="padk")
                nc.vector.tensor_scalar(pad_k[:], mcnt[:], -1.0, float(TOPK),
                                        op0=OP.mult, op1=OP.add)
                # the (8-m) padding positions (lowest-index non-mutual) always
                # sit within the first 16 columns (<=8 mutual + <=8 pads), so
                # scan/select only the 16-column head; beyond that sel == M.
                s = pool.tile([128, 16], bf16, name="scan", tag="scan", bufs=2)
                nc.vector.tensor_tensor_scan(
                    out=s[:], data0=ones_bf[:, 0:16], data1=M[:, 0:16], initial=0.0,
                    op0=OP.add, op1=OP.subtract)
                sel = pool.tile([128, 16], bf16, name="sel", tag="sel", bufs=2)
                nc.vector.scalar_tensor_tensor(
                    out=sel[:], in0=s[:], scalar=pad_k[:], in1=M[:, 0:16],
                    op0=OP.is_le, op1=OP.max)
                if debug and dump:
                    nc.gpsimd.dma_start(dbg_M[ts(m, 128), :], M[:])
                    nc.gpsimd.dma_start(dbg_sel[ts(m, 128), :], sel[:])
                    nc.sync.dma_start(dbg_mcnt[ts(m, 128), :], mcnt[:])
                e = pool.tile([128, B], f16, name="ee", tag="ee", bufs=2)
                nc.scalar.activation(e[:], srow[:], AF.Exp, bias=nmax_tile[:, m:m + 1])
                P = pool.tile([128, B], f16, name="pp", tag="pp", bufs=1)
                den_a = pool.tile([128, 1], f32, name="dena", tag="dena")
                den_b = pool.tile([128, 1], f32, name="denb", tag="denb")
                nc.vector.scalar_tensor_tensor(
                    out=P[:, 0:16], in0=e[:, 0:16], scalar=1.0, in1=sel[:],
                    op0=OP.mult, op1=OP.mult, accum_out=den_a[:])
                nc.vector.scalar_tensor_tensor(
                    out=P[:, 16:B], in0=e[:, 16:B], scalar=1.0, in1=M[:, 16:B],
                    op0=OP.mult, op1=OP.mult, accum_out=den_b[:])
                den = pool.tile([128, 1], f32, name="den", tag="den")
                nc.vector.tensor_add(den[:], den_a[:], den_b[:])
                winv = pool.tile([128, 1], f32, name="winv", tag="winv")
                nc.vector.reciprocal(winv[:], den[:])
                wrow = pool.tile([128, B], f16, name=f"wrow{m}", tag=f"wrow{m}")
                wsum = pool.tile([128, 1], f32, name="wsum", tag="wsum")
                nc.scalar.activation(wrow[:], P[:], AF.Copy, scale=winv[:],
                                     accum_out=wsum[:])
                # de_inv = 1 / (wsum + 2 + eps)
                det = pool.tile([128, 1], f32, name="det", tag="det")
                nc.vector.tensor_scalar(det[:], wsum[:], SELF_LOOP + EPS_DEG, None,
                                        op0=OP.add)
                nc.vector.reciprocal(de_tile[:, m:m + 1], det[:])
                nc.sync.dma_start(w_dram[ts(m, 128), :], wrow[:])
                wrows.append(wrow)
            # transpose W rows -> wT (B x CH), accumulate colsums
            for tt in range(NT):
                ps = psp.tile([128, 512], f16, name="wtps", tag="wtps")
                for j in range(NTC):
                    nc.tensor.transpose(
                        ps[:, ts(j, 128)], wrows[j][:, ts(tt, 128)], id_f16[:])
                stg = pool.tile([128, 512], f16, name="wtstg", tag="wtstg", bufs=3)
                nc.scalar.activation(stg[:], ps[:], AF.Copy,
                                     accum_out=colp_tile[:, tt:tt + 1])
                nc.sync.dma_start(wT_dram[ts(tt, 128), :], stg[:])

        with tc.tile_pool(name="a4", bufs=1) as a4, \
             tc.tile_pool(name="a4ps", bufs=4, space="PSUM") as a4ps:
            build_W(S_dram, thrA_my, nmaxA, load_thr_bc(1), Wt_dram, WtT_dram,
                    de_inv_i, colp_txt, a4, a4ps, dump=True)
            build_W(St_dram, thrB_my, nmaxB, load_thr_bc(0), Wi_dram, WiT_dram,
                    de_inv_t, colp_img, a4, a4ps)

        thrb_pool.release()

        if debug:
            nc.sync.dma_start(dbg_S[:, :], S_dram[:, :])
            nc.sync.dma_start(dbg_wt[:, :], Wt_dram[:, :])
            nc.sync.dma_start(dbg_wi[:, :], Wi_dram[:, :])
            nc.sync.dma_start(dbg_thr[:, :], thr_out[:, :])
            nc.sync.dma_start(dbg_dv[:, :], dv_ar_out[:, :])

        # ---- phase-B persistents (allocated after phase-A pools freed) ----
        persist2 = tc.alloc_tile_pool(name="persist2", bufs=1)
        xchunk = [[persist2.tile([128, D], f32, name=f"xc{sl}_{t}", tag=f"xc{sl}_{t}") for t in range(8)]
                  for sl in range(2)]
        g_l = [persist2.tile([128, D], f32, name=f"g{l}", tag=f"g{l}") for l in range(LAYERS)]
        b_l = [persist2.tile([128, D], f32, name=f"b{l}", tag=f"b{l}") for l in range(LAYERS)]
        g_fi = persist2.tile([128, D], f32, name="gfi", tag="gfi")
        b_fi = persist2.tile([128, D], f32, name="bfi", tag="bfi")
        g_ft = persist2.tile([128, D], f32, name="gft", tag="gft")
        b_ft = persist2.tile([128, D], f32, name="bft", tag="bft")
        for l in range(LAYERS):
            nc.sync.dma_start(g_l[l][:], ln_g[l:l + 1, :].to_broadcast((128, D)))
            nc.sync.dma_start(b_l[l][:], ln_b[l:l + 1, :].to_broadcast((128, D)))
        nc.sync.dma_start(g_fi[:], ln_img_g[:].rearrange("(o d) -> o d", o=1).to_broadcast((128, D)))
        nc.sync.dma_start(b_fi[:], ln_img_b[:].rearrange("(o d) -> o d", o=1).to_broadcast((128, D)))
        nc.sync.dma_start(g_ft[:], ln_txt_g[:].rearrange("(o d) -> o d", o=1).to_broadcast((128, D)))
        nc.sync.dma_start(b_ft[:], ln_txt_b[:].rearrange("(o d) -> o d", o=1).to_broadcast((128, D)))

        # ---------- A7: degree collectives ----------
        with tc.tile_pool(name="a7", bufs=1) as a7:
            arin = a7.tile([128, 64], f32, name="arin", tag="arin")
            nc.vector.tensor_copy(arin[:, 0:NT], colp_img[:])
            nc.vector.tensor_copy(arin[:, NT:2 * NT], colp_txt[:])
            nc.sync.dma_start(dv_ar_in[:, :], arin[:])
            nc.gpsimd.collective_compute(
                "AllReduce", OP.add, replica_groups=RG,
                ins=[dv_ar_in[:, :].opt()], outs=[dv_ar_out[:, :].opt()])
            for cp in range(NCORES):
                nc.sync.dma_start(dv_rs_in[cp, :, 0:4], arin[:, 4 * cp:4 * cp + 4])
                nc.sync.dma_start(dv_rs_in[cp, :, 4:8],
                                  arin[:, NT + 4 * cp:NT + 4 * cp + 4])
            nc.gpsimd.collective_compute(
                "ReduceScatter", OP.add, replica_groups=RG,
                ins=[dv_rs_in[:, :, :].opt()], outs=[dv_rs_out[:, :, :].opt()])
            # dv_is = 1/sqrt(colsum + 2 + eps)
            dvf = a7.tile([128, 64], f32, name="dvf", tag="dvf")
            nc.sync.dma_start(dvf[:], dv_ar_out[:, :])
            dvb = a7.tile([128, 64], f32, name="dvb", tag="dvb")
            nc.vector.tensor_scalar(dvb[:], dvf[:], SELF_LOOP + EPS_DEG, None, op0=OP.add)
            dsq = a7.tile([128, 64], f32, name="dsq", tag="dsq")
            nc.scalar.activation(dsq[:], dvb[:], AF.Sqrt)
            nc.vector.reciprocal(dv_is_full[:], dsq[:])
            dvm = a7.tile([128, 8], f32, name="dvm", tag="dvm")
            nc.sync.dma_start(dvm[:], dv_rs_out[0, :, :])
            dmb = a7.tile([128, 8], f32, name="dmb", tag="dmb")
            nc.vector.tensor_scalar(dmb[:], dvm[:], SELF_LOOP + EPS_DEG, None, op0=OP.add)
            dmq = a7.tile([128, 8], f32, name="dmq", tag="dmq")
            nc.scalar.activation(dmq[:], dmb[:], AF.Sqrt)
            nc.vector.reciprocal(dv_is_my[:], dmq[:])
            if debug:
                nc.sync.dma_start(dbg_de[:, 0:4], de_inv_i[:])
                nc.sync.dma_start(dbg_de[:, 4:8], de_inv_t[:])

        # ---------- B: layers ----------
        # init X chunk from inputs
        with tc.tile_pool(name="binit", bufs=2) as bi:
            for t in range(NTC):
                nc.sync.dma_start(xchunk[0][t][:], f_img_my[ts(t, 128), :])
                nc.sync.dma_start(xchunk[0][4 + t][:], f_txt_my[ts(t, 128), :])

        for layer in range(LAYERS):
            xc_in = xchunk[layer % 2]
            xc_out = xchunk[(layer + 1) % 2]
            lp = tc.alloc_tile_pool(name=f"L{layer}", bufs=1)
            # X0 full (fp16): img tiles / txt tiles
            x0i = [lp.tile([128, D], f16, name=f"x0i{t}", tag=f"x0i{t}") for t in range(NT)]
            x0t = [lp.tile([128, D], f16, name=f"x0t{t}", tag=f"x0t{t}") for t in range(NT)]
            with tc.tile_pool(name=f"bx{layer}", bufs=4) as bx:
                for t in range(NT):
                    sti = bx.tile([128, D], f16 if layer else f32, name="xsti", tag="xsti")
                    stt = bx.tile([128, D], f16 if layer else f32, name="xstt", tag="xstt")
                    if layer == 0:
                        nc.sync.dma_start(sti[:], f_img[ts(t, 128), :])
                        nc.sync.dma_start(stt[:], f_txt[ts(t, 128), :])
                    else:
                        cp_, j_ = t // NTC, t % NTC
                        base = cp_ * 2 * CH
                        nc.sync.dma_start(
                            sti[:], xag_out[base + j_ * 128: base + (j_ + 1) * 128, :])
                        nc.sync.dma_start(
                            stt[:], xag_out[base + CH + j_ * 128:
                                            base + CH + (j_ + 1) * 128, :])
                    nc.scalar.activation(x0i[t][:], sti[:], AF.Copy,
                                         scale=dv_is_full[:, t:t + 1])
                    nc.scalar.activation(x0t[t][:], stt[:], AF.Copy,
                                         scale=dv_is_full[:, NT + t:NT + t + 1])
            # X0 of my chunk (fp16)
            x0mi = [lp.tile([128, D], f16, name=f"x0mi{t}", tag=f"x0mi{t}") for t in range(NTC)]
            x0mt = [lp.tile([128, D], f16, name=f"x0mt{t}", tag=f"x0mt{t}") for t in range(NTC)]
            for t in range(NTC):
                nc.scalar.activation(x0mi[t][:], xc_in[t][:], AF.Copy,
                                     scale=dv_is_my[:, t:t + 1])
                nc.scalar.activation(x0mt[t][:], xc_in[4 + t][:], AF.Copy,
                                     scale=dv_is_my[:, 4 + t:4 + t + 1])

            # Xe = (2*X0_my + W @ X0_other) * de_inv     (my edge rows)
            xei = [lp.tile([128, D], f16, name=f"xei{t}", tag=f"xei{t}") for t in range(NTC)]
            xet = [lp.tile([128, D], f16, name=f"xet{t}", tag=f"xet{t}") for t in range(NTC)]

            def edge_feats(wT_d, x0_other, x0_my, de_tile, xe, pool, psp):
                pss = [psp.tile([128, D], f32, name=f"xeps{m}", tag=f"xeps{m}", bufs=1) for m in range(NTC)]
                for k in range(NT):
                    kxm = pool.tile([128, CH], f16, name="wTk", tag="wTk", bufs=6)
                    nc.sync.dma_start(kxm[:], wT_d[ts(k, 128), :])
                    for m in range(NTC):
                        nc.tensor.matmul(pss[m][:], kxm[:, ts(m, 128)], x0_other[k][:],
                                         start=(k == 0), stop=(k == NT - 1))
                for m in range(NTC):
                    tmp = pool.tile([128, D], f32, name="xetmp", tag="xetmp", bufs=2)
                    nc.vector.scalar_tensor_tensor(
                        out=tmp[:], in0=x0_my[m][:], scalar=2.0, in1=pss[m][:],
                        op0=OP.mult, op1=OP.add)
                    nc.scalar.activation(xe[m][:], tmp[:], AF.Copy,
                                         scale=de_tile[:, m:m + 1])

            with tc.tile_pool(name=f"bxe{layer}", bufs=2) as bxe, \
                 tc.tile_pool(name=f"bxeps{layer}", bufs=1, space="PSUM") as bxeps:
                edge_feats(WtT_dram, x0t, x0mi, de_inv_i, xei, bxe, bxeps)
                edge_feats(WiT_dram, x0i, x0mt, de_inv_t, xet, bxe, bxeps)

            # X1 partials: img_part = Wi^T-chunk contribution, txt likewise
            def node_partials(w_d, xe_src, img_side, pool, psp):
                # partial[:, :] += w_d[kslice, mslice]^T @ xe ; write to x1p slots
                for g in range(NT // 8):      # groups of 8 m-tiles -> 8 psum banks
                    pss = [psp.tile([128, D], f32, name=f"x1ps{j}", tag=f"x1ps{j}",
                                    bufs=1) for j in range(8)]
                    for k in range(NTC):
                        kxm = pool.tile([128, 1024], f16, name="wkb", tag="wkb", bufs=3)
                        nc.sync.dma_start(kxm[:], w_d[ts(k, 128), g * 1024:(g + 1) * 1024])
                        for j in range(8):
                            nc.tensor.matmul(pss[j][:], kxm[:, ts(j, 128)], xe_src[k][:],
                                             start=(k == 0), stop=(k == NTC - 1))
                    for j in range(8):
                        m = g * 8 + j
                        stg = pool.tile([128, D], f16, name="x1stg", tag="x1stg", bufs=4)
                        nc.scalar.copy(stg[:], pss[j][:])
                        cp_, j_ = m // NTC, m % NTC
                        off = j_ * 128 if img_side else CH + j_ * 128
                        nc.sync.dma_start(x1p[layer][cp_, off:off + 128, :], stg[:])

            with tc.tile_pool(name=f"bx1{layer}", bufs=2) as bx1, \
                 tc.tile_pool(name=f"bx1ps{layer}", bufs=1, space="PSUM") as bx1ps:
                node_partials(Wi_dram, xet, True, bx1, bx1ps)    # -> img nodes
                node_partials(Wt_dram, xei, False, bx1, bx1ps)   # -> txt nodes

            nc.gpsimd.collective_compute(
                "ReduceScatter", OP.add, replica_groups=RG,
                ins=[x1p[layer][:, :, :].opt()], outs=[x1rs[layer][:, :, :].opt()])

            # X1 chunk = (rs + 2*Xe_my) * dv_is_my ; FC; residual+LN
            with tc.tile_pool(name=f"bfc{layer}", bufs=2) as bf, \
                 tc.tile_pool(name=f"bfcps{layer}", bufs=4, space="PSUM") as bfp:
                x1t_ = [bf.tile([128, 2 * CH], f16, name=f"x1T{k}", tag=f"x1T{k}", bufs=1)
                        for k in range(ND)]
                x1tiles = []
                for t in range(8):
                    rst = bf.tile([128, D], f16, name="rst", tag="rst", bufs=3)
                    nc.sync.dma_start(rst[:], x1rs[layer][0, ts(t, 128), :])
                    xe_my = (xei if t < 4 else xet)[t % 4]
                    tmp = bf.tile([128, D], f32, name="x1tmp", tag="x1tmp", bufs=2)
                    nc.vector.scalar_tensor_tensor(
                        out=tmp[:], in0=xe_my[:], scalar=2.0, in1=rst[:],
                        op0=OP.mult, op1=OP.add)
                    x1c = bf.tile([128, D], f16, name=f"x1c{t}", tag=f"x1c{t}", bufs=1)
                    nc.scalar.activation(x1c[:], tmp[:], AF.Copy,
                                         scale=dv_is_my[:, t:t + 1])
                    x1tiles.append(x1c)
                    if debug and layer == 0:
                        nc.sync.dma_start(dbg_x1[ts(t, 128), :], x1c[:])
                        nc.sync.dma_start(dbg_xe[ts(t, 128), :], xe_my[:])
                # transpose x1 (1024 x 512) -> x1T (512 x 1024)
                for k in range(ND):
                    for half in range(2):
                        ps = bfp.tile([128, D], f16, name="x1tps", tag="x1tps", bufs=2)
                        for j in range(4):
                            t = half * 4 + j
                            nc.tensor.transpose(
                                ps[:, ts(j, 128)], x1tiles[t][:, ts(k, 128)], id_f16[:])
                        nc.vector.tensor_copy(x1t_[k][:, ts(half, 512)], ps[:])
                # fc = gelu(x1 @ Wfc^T); residual + LN per node tile
                for t in range(8):
                    ps = bfp.tile([128, D], f32, name="fcps", tag="fcps", bufs=2)
                    for k in range(ND):
                        nc.tensor.matmul(ps[:], x1t_[k][:, ts(t, 128)], wt_fc[layer][k][:],
                                         start=(k == 0), stop=(k == ND - 1))
                    fc = bf.tile([128, D], f32, name="fc", tag="fc", bufs=2)
                    nc.scalar.activation(fc[:], ps[:], AF.Gelu)
                    r = bf.tile([128, D], f32, name="rres", tag="rres", bufs=2)
                    nc.vector.tensor_add(r[:], xc_in[t][:], fc[:])
                    _layer_norm(nc, bf, r, g_l[layer], b_l[layer], xc_out[t])
                    st16 = bf.tile([128, D], f16, name="st16", tag="st16", bufs=2)
                    nc.scalar.copy(st16[:], xc_out[t][:])
                    if layer + 1 < LAYERS:
                        nc.sync.dma_start(xag_in[ts(t, 128), :], st16[:])
            if layer + 1 < LAYERS:
                nc.gpsimd.collective_compute(
                    "AllGather", OP.bypass, replica_groups=RG,
                    ins=[xag_in[:, :].opt()], outs=[xag_out[:, :].opt()])
            lp.release()

        # ---------- final per-modality LN ----------
        with tc.tile_pool(name="fin", bufs=2) as fin:
            xf = xchunk[LAYERS % 2]
            for t in range(8):
                o = fin.tile([128, D], f32, name="fino", tag="fino", bufs=2)
                g, b_ = (g_fi, b_fi) if t < 4 else (g_ft, b_ft)
                _layer_norm(nc, fin, xf[t], g, b_, o)
                if t < 4:
                    nc.sync.dma_start(out_img_my[ts(t, 128), :], o[:])
                else:
                    nc.sync.dma_start(out_txt_my[ts(t % 4, 128), :], o[:])

        persist2.release()
        const.release()

    global PREDICTED_NS
    try:
        pe = getattr(tc, "_perfetto_entries", None)
        if pe:
            PREDICTED_NS = max(e[2] for e in pe) - min(e[1] for e in pe)
    except Exception:
        pass
    _fixup_multi_waits(nc)
    return nc


def _layer_norm(nc, pool, x, g_bc, b_bc, out):
    f32 = dt.float32
    s = pool.tile([128, 1], f32, name="lns", tag="lns")
    nc.vector.tensor_reduce(s[:], x[:], axis=mybir.AxisListType.X, op=OP.add)
    mu = pool.tile([128, 1], f32, name="lnmu", tag="lnmu")
    nc.vector.tensor_scalar(mu[:], s[:], 1.0 / D, None, op0=OP.mult)
    xc = pool.tile([128, D], f32, name="lnxc", tag="lnxc", bufs=2)
    nc.vector.tensor_scalar(xc[:], x[:], mu[:], None, op0=OP.subtract)
    sq = pool.tile([128, D], f32, name="lnsq", tag="lnsq", bufs=2)
    ssq = pool.tile([128, 1], f32, name="lnssq", tag="lnssq")
    nc.scalar.activation(sq[:], xc[:], AF.Square, accum_out=ssq[:])
    var = pool.tile([128, 1], f32, name="lnvar", tag="lnvar")
    nc.vector.tensor_scalar(var[:], ssq[:], 1.0 / D, None, op0=OP.mult)
    vb = pool.tile([128, 1], f32, name="lnvb", tag="lnvb")
    nc.vector.tensor_scalar(vb[:], var[:], LN_EPS, None, op0=OP.add)
    sd = pool.tile([128, 1], f32, name="lnsd", tag="lnsd")
    nc.scalar.activation(sd[:], vb[:], AF.Sqrt)
    rstd = pool.tile([128, 1], f32, name="lnrstd", tag="lnrstd")
    nc.vector.reciprocal(rstd[:], sd[:])
    xn = pool.tile([128, D], f32, name="lnxn", tag="lnxn", bufs=2)
    nc.vector.scalar_tensor_tensor(
        out=xn[:], in0=xc[:], scalar=rstd[:], in1=g_bc[:], op0=OP.mult, op1=OP.mult)
    nc.vector.tensor_add(out[:], xn[:], b_bc[:])


_NC_CACHE = None
TRACE = False
LAST_EXEC_NS = None
PREDICTED_NS = None


def _get_nc():
    global _NC_CACHE
    if _NC_CACHE is None:
        _NC_CACHE = build_nc()
    return _NC_CACHE


def kernel(f_img, f_txt, W, ln_g, ln_b, ln_img_g, ln_img_b, ln_txt_g, ln_txt_b):
    f_img = np.ascontiguousarray(np.asarray(f_img, dtype=np.float32))
    f_txt = np.ascontiguousarray(np.asarray(f_txt, dtype=np.float32))
    arrs = dict(
        f_img=f_img, f_txt=f_txt,
        Wfc=np.ascontiguousarray(np.asarray(W, dtype=np.float32)),
        ln_g=np.ascontiguousarray(np.asarray(ln_g, dtype=np.float32)),
        ln_b=np.ascontiguousarray(np.asarray(ln_b, dtype=np.float32)),
        ln_img_g=np.ascontiguousarray(np.asarray(ln_img_g, dtype=np.float32)),
        ln_img_b=np.ascontiguousarray(np.asarray(ln_img_b, dtype=np.float32)),
        ln_txt_g=np.ascontiguousarray(np.asarray(ln_txt_g, dtype=np.float32)),
        ln_txt_b=np.ascontiguousarray(np.asarray(ln_txt_b, dtype=np.float32)),
    )
    in_maps = []
    for c in range(NCORES):
        m = dict(arrs)
        m["f_img_my"] = np.ascontiguousarray(f_img[c * CH:(c + 1) * CH])
        m["f_txt_my"] = np.ascontiguousarray(f_txt[c * CH:(c + 1) * CH])
        in_maps.append(m)
    global LAST_EXEC_NS
    res = bass_utils.run_bass_kernel_spmd(
        _get_nc(), in_maps, core_ids=list(range(NCORES)), trace=TRACE)
    LAST_EXEC_NS = res.exec_time_ns
    out_img = np.concatenate([res.results[c]["out_img_my"] for c in range(NCORES)], 0)
    out_txt = np.concatenate([res.results[c]["out_txt_my"] for c in range(NCORES)], 0)
    return (out_img, out_txt)

